# revision 1
# baseline (speedup 1.0000x reference)
"""Multi-head causal attention (B=2, S=2048, D=1024, H=16) on 8 trn2 cores.

Sharding: core c -> batch b=c//4, head-group g=c%4 (heads 4g..4g+3).
Each core: Q/K/V projections for its heads from xT[b], causal attention in
transposed layout, row-parallel out-projection partial. Host sums the 4
partials per batch (bias is fed as zeros to 3 of the 4 cores and applied
on-device via a K=1 broadcast matmul).

All matmul-feeding tensors are float32r: 4-byte fp32 storage that the PE
streams at 1 cycle/row (strict fp32 is 4x slower).
"""

import numpy as np

import concourse.bass as bass
import concourse.tile as tile
import concourse.mybir as mybir
from concourse import bacc
from concourse.bass_utils import run_bass_kernel_spmd

B, S, D, H, DH = 2, 2048, 1024, 16, 64
NCORES = 8
HPC = 4          # heads per core
PAIRS = 2        # head pairs per core
QT = 512         # q tile (free dim of scoresT / PV matmuls)
KB = 128         # k block (partition dim of scoresT)
NQT = S // QT    # 4
NKB = S // KB    # 16
DC = D // 128    # 8 contraction chunks for projections
SCALE = 1.0 / np.sqrt(DH)

F32 = mybir.dt.float32
FR = mybir.dt.float32r
BF = mybir.dt.bfloat16

# bench-only ablation switches (set of strings), see profile_wall.py
ABLATE = set()


def _build(reps=None):
    """reps: if set, wrap the whole body in an on-device For_i loop (bench only)."""
    import contextlib
    nc = bacc.Bacc("TRN2", target_bir_lowering=False, debug=False, num_devices=NCORES)

    xT = nc.dram_tensor("xT", [D, S], BF, kind="ExternalInput").ap()
    wq = nc.dram_tensor("wq", [D, HPC * DH], BF, kind="ExternalInput").ap()
    wk = nc.dram_tensor("wk", [D, HPC * DH], BF, kind="ExternalInput").ap()
    wv = nc.dram_tensor("wv", [D, HPC * DH], BF, kind="ExternalInput").ap()
    wo = nc.dram_tensor("wo", [HPC * DH, D], BF, kind="ExternalInput").ap()
    bo_r = nc.dram_tensor("bo_r", [1, D], BF, kind="ExternalInput").ap()
    tri = nc.dram_tensor("tri", [KB, KB], BF, kind="ExternalInput").ap()
    out = nc.dram_tensor("out", [S, D], F32, kind="ExternalOutput").ap()

    with tile.TileContext(nc) as tc, \
         (tc.For_i(0, reps, 1) if reps else contextlib.nullcontext()), \
         tc.tile_pool(name="persist", bufs=1) as persist:
        # ---- persistent tiles ----
        qt_sb = [persist.tile([128, S], BF, name=f"qt{p}", tag=f"qt{p}") for p in range(PAIRS)]
        kt_sb = [persist.tile([128, S], BF, name=f"kt{p}", tag=f"kt{p}") for p in range(PAIRS)]
        # V' tiles: per s-block j, [128, 4*65]; head hl at cols 65*hl, ones col at 65*hl+64
        vt_sb = [persist.tile([128, HPC * (DH + 1)], BF, name=f"vt{j}", tag=f"vt{j}") for j in range(NKB)]
        ctx_sb = [persist.tile([128, S], BF, name=f"ctx{p}", tag=f"ctx{p}") for p in range(PAIRS)]
        wo_sb = [persist.tile([128, D], BF, name=f"wo{p}", tag=f"wo{p}") for p in range(PAIRS)]
        tri_sb = persist.tile([KB, KB], BF, name="tri", tag="tri")
        bo_sb = persist.tile([1, D], BF, name="bo", tag="bo")
        ones_sb = persist.tile([1, 128], BF, name="ones", tag="ones")

        xts = [persist.tile([128, S], BF, name=f"xts{i}", tag=f"xts{i}") for i in range(DC)]
        wq_sb = [persist.tile([128, HPC * DH], BF, name=f"wq{i}", tag=f"wq{i}") for i in range(DC)]
        wk_sb = [persist.tile([128, HPC * DH], BF, name=f"wk{i}", tag=f"wk{i}") for i in range(DC)]
        wv_sb = [persist.tile([128, HPC * DH], BF, name=f"wv{i}", tag=f"wv{i}") for i in range(DC)]

        nc.sync.dma_start(tri_sb[:], tri[:])
        nc.sync.dma_start(bo_sb[:], bo_r[:])
        nc.gpsimd.memset(ones_sb[:], 1.0)
        for i in range(DC):
            nc.sync.dma_start(xts[i][:], xT[i * 128:(i + 1) * 128, :])
            nc.sync.dma_start(wq_sb[i][:], wq[i * 128:(i + 1) * 128, :])
            nc.sync.dma_start(wk_sb[i][:], wk[i * 128:(i + 1) * 128, :])
            nc.sync.dma_start(wv_sb[i][:], wv[i * 128:(i + 1) * 128, :])
        for p in range(PAIRS):
            nc.sync.dma_start(wo_sb[p][:], wo[p * 128:(p + 1) * 128, :])

        def proj_qk_chunked(p, pool):
            """q/k projection for pair p, D-chunk-outer so matmuls chase the
            xT DMAs chunk by chunk. Holds 8 psum banks."""
            qps = [pool.tile([128, QT], F32, name=f"qps{st}", tag=f"qk{st}") for st in range(NQT)]
            kps = [pool.tile([128, QT], F32, name=f"kps{st}", tag=f"qk{4 + st}") for st in range(NQT)]
            for i in range(DC):
                for st in range(NQT):
                    nc.tensor.matmul(
                        qps[st][:], wq_sb[i][:, p * 128:(p + 1) * 128],
                        xts[i][:, st * QT:(st + 1) * QT],
                        start=(i == 0), stop=(i == DC - 1))
                for st in range(NQT):
                    nc.tensor.matmul(
                        kps[st][:], wk_sb[i][:, p * 128:(p + 1) * 128],
                        xts[i][:, st * QT:(st + 1) * QT],
                        start=(i == 0), stop=(i == DC - 1))
            for st in range(NQT):
                nc.scalar.copy(qt_sb[p][:, st * QT:(st + 1) * QT], qps[st][:])
                nc.vector.tensor_copy(kt_sb[p][:, st * QT:(st + 1) * QT], kps[st][:])

        def proj_qk_seq(p, pool):
            """q/k projection, sequential psum (2 banks) — for overlap with
            attention of the other pair."""
            for st in range(NQT):
                qp = pool.tile([128, QT], F32, name="qp", tag="qkseq")
                for i in range(DC):
                    nc.tensor.matmul(
                        qp[:], wq_sb[i][:, p * 128:(p + 1) * 128],
                        xts[i][:, st * QT:(st + 1) * QT],
                        start=(i == 0), stop=(i == DC - 1))
                nc.scalar.copy(qt_sb[p][:, st * QT:(st + 1) * QT], qp[:])
                kp = pool.tile([128, QT], F32, name="kp", tag="qkseq")
                for i in range(DC):
                    nc.tensor.matmul(
                        kp[:], wk_sb[i][:, p * 128:(p + 1) * 128],
                        xts[i][:, st * QT:(st + 1) * QT],
                        start=(i == 0), stop=(i == DC - 1))
                nc.vector.tensor_copy(kt_sb[p][:, st * QT:(st + 1) * QT], kp[:])

        def attention(p, h, qt_i, scps, ctxps, att, attsm):
            hl = 2 * p + h
            r0, r1 = h * 64, h * 64 + 64
            q0 = qt_i * QT
            nkb = 4 * (qt_i + 1)
            cps = ctxps.tile([DH + 1, QT], F32, name="cps", tag="cps")
            for g0 in range(0, nkb, 2):
                sp = scps.tile([128, 2 * QT], F32, name="sp", tag="sp")
                for u in range(2):
                    kb = g0 + u
                    nc.tensor.matmul(
                        sp[:, u * QT:(u + 1) * QT],
                        kt_sb[p][r0:r1, kb * KB:(kb + 1) * KB],
                        qt_sb[p][r0:r1, q0:q0 + QT],
                        start=True, stop=True)
                pt = att.tile([128, 2 * QT], BF, name="pt", tag="pt")
                nc.scalar.activation(
                    pt[:], sp[:], mybir.ActivationFunctionType.Exp,
                    scale=float(SCALE))
                # causal masking: straddle groups are exactly g0==4qt (d=0,1)
                # and g0==4qt+2 (d=2,3): memset the dead rectangles (gpsimd),
                # multiply the [128,128] diagonal triangles (DVE)
                if "masks" in ABLATE:
                    pass
                elif g0 == 4 * qt_i:          # blocks d=0, d=1
                    nc.gpsimd.memset(pt[:, QT:QT + KB], 0.0)
                    for off in (0, QT + KB):
                        nc.vector.tensor_mul(
                            pt[:, off:off + KB], pt[:, off:off + KB], tri_sb[:])
                elif g0 == 4 * qt_i + 2:    # blocks d=2, d=3
                    nc.gpsimd.memset(pt[:, 0:2 * KB], 0.0)
                    nc.gpsimd.memset(pt[:, QT:QT + 3 * KB], 0.0)
                    for off in (2 * KB, QT + 3 * KB):
                        nc.vector.tensor_mul(
                            pt[:, off:off + KB], pt[:, off:off + KB], tri_sb[:])
                for u in range(2):
                    kb = g0 + u
                    nc.tensor.matmul(
                        cps[:],
                        vt_sb[kb][:, hl * (DH + 1):(hl + 1) * (DH + 1)],
                        pt[:, u * QT:(u + 1) * QT],
                        start=(kb == 0), stop=(kb == nkb - 1))
            # normalize: r = 1/l broadcast over the 64 ctx rows
            r_sb = attsm.tile([1, QT], F32, name="r_sb", tag="r")
            if "recip" in ABLATE:
                nc.vector.tensor_copy(r_sb[:], cps[DH:DH + 1, :])
            else:
                nc.vector.reciprocal(r_sb[:], cps[DH:DH + 1, :])
            rb = attsm.tile([64, QT], F32, name="rb", tag="rb")
            nc.gpsimd.partition_broadcast(rb[:], r_sb[:])
            nc.vector.tensor_mul(
                ctx_sb[p][r0:r1, q0:q0 + QT], cps[0:DH, :], rb[:])

        def outproj(qt_i, ph3ps, ph3sb):
            """partial out-projection rows for one q tile, bias folded in as a
            K=1 matmul; copy PSUM->SBUF split across ACT/DVE, then DMA out."""
            for qb in range(qt_i * 4, qt_i * 4 + 4):
                os_ = ph3sb.tile([128, D], F32, name="os", tag="os")
                for nh in range(2):
                    op = ph3ps.tile([128, 512], F32, name="op", tag="op")
                    nc.tensor.matmul(
                        op[:], ones_sb[:], bo_sb[:, nh * 512:(nh + 1) * 512],
                        start=True, stop=False)
                    for p in range(PAIRS):
                        nc.tensor.matmul(
                            op[:], ctx_sb[p][:, qb * 128:(qb + 1) * 128],
                            wo_sb[p][:, nh * 512:(nh + 1) * 512],
                            start=False, stop=(p == PAIRS - 1))
                    if "outio" in ABLATE:
                        continue
                    dst = os_[:, nh * 512:(nh + 1) * 512]
                    if nh == 0:
                        nc.scalar.copy(dst, op[:])
                    else:
                        nc.vector.tensor_copy(dst, op[:])
                if "outdma" not in ABLATE and "outio" not in ABLATE:
                    nc.sync.dma_start(out[qb * 128:(qb + 1) * 128, :], os_[:])

        # phase A: q/k pair 0, chunk-pipelined against the input DMAs
        with tc.tile_pool(name="qk0ps", bufs=1, space="PSUM") as qk0ps:
            proj_qk_chunked(0, qk0ps)

        # phase B onwards: V (2 psum banks) + attention pools (6 banks)
        with tc.tile_pool(name="att", bufs=4) as att, \
             tc.tile_pool(name="attsm", bufs=4) as attsm, \
             tc.tile_pool(name="scps", bufs=2, space="PSUM") as scps, \
             tc.tile_pool(name="ctxps", bufs=2, space="PSUM") as ctxps:

            skip_attn = "attn" in ABLATE
            with tc.tile_pool(name="vps", bufs=2, space="PSUM") as vps:
                for j in range(NKB):
                    vp = vps.tile([128, HPC * DH], F32, name="vp", tag="vp")
                    for i in range(DC):
                        nc.tensor.matmul(
                            vp[:], xts[i][:, j * 128:(j + 1) * 128], wv_sb[i][:],
                            start=(i == 0), stop=(i == DC - 1))
                    vt_view = vt_sb[j].rearrange("p (h e) -> p h e", h=HPC)
                    nc.vector.tensor_copy(
                        vt_view[:, :, 0:DH], vp.rearrange("p (h e) -> p h e", h=HPC))
                    nc.gpsimd.memset(vt_view[:, :, DH:DH + 1], 1.0)

                # pair-0 attention (starts as soon as early vt tiles land)
                for qt_i in range(NQT):
                    for h in range(2):
                        if not skip_attn:
                            attention(0, h, qt_i, scps, ctxps, att, attsm)

            # q/k pair 1 hides under pair-0 attention's ACT/DVE span
            with tc.tile_pool(name="qk1ps", bufs=2, space="PSUM") as qk1ps:
                proj_qk_seq(1, qk1ps)

            # pair-1 attention, out-projection interleaved per finished q tile
            with tc.tile_pool(name="ph3ps", bufs=2, space="PSUM") as ph3ps, \
                 tc.tile_pool(name="ph3sb", bufs=3) as ph3sb:
                for qt_i in range(NQT):
                    for h in range(2):
                        if not skip_attn:
                            attention(1, h, qt_i, scps, ctxps, att, attsm)
                    if "outproj" not in ABLATE and not skip_attn and qt_i > 0:
                        outproj(qt_i - 1, ph3ps, ph3sb)
                if "outproj" not in ABLATE and not skip_attn:
                    outproj(NQT - 1, ph3ps, ph3sb)

    nc.compile()
    return nc


_NC = None
PROFILE = False
TRACE_CORES = (0,)
LAST_RESULT = None


def _get_nc():
    global _NC
    if _NC is None:
        _NC = _build()
    return _NC


def kernel(x, Wq, Wk, Wv, Wo, bo):
    x = np.asarray(x, dtype=np.float32)
    Wq = np.asarray(Wq, dtype=np.float32)
    Wk = np.asarray(Wk, dtype=np.float32)
    Wv = np.asarray(Wv, dtype=np.float32)
    Wo = np.asarray(Wo, dtype=np.float32)
    bo = np.asarray(bo, dtype=np.float32)

    nc = _get_nc()

    in_maps = _prepare_in_maps(x, Wq, Wk, Wv, Wo, bo)

    global LAST_RESULT
    kw = {}
    if PROFILE:
        kw = dict(trace=True, trace_cores=list(TRACE_CORES))
    res = run_bass_kernel_spmd(nc, in_maps, core_ids=list(range(NCORES)), **kw)
    LAST_RESULT = res

    out = np.zeros((B, S, D), np.float32)
    for c in range(NCORES):
        b = c // 4
        out[b] += res.results[c]["out"]
    return out


def _prepare_in_maps(x, Wq, Wk, Wv, Wo, bo):
    kk = np.arange(KB)[:, None]
    qq = np.arange(KB)[None, :]
    import ml_dtypes
    tri = (kk <= qq).astype(ml_dtypes.bfloat16)

    bf16 = ml_dtypes.bfloat16
    xTs = [np.ascontiguousarray(x[b].T).astype(bf16) for b in range(B)]
    bo_row = np.ascontiguousarray(bo[None, :]).astype(bf16)
    zeros_row = np.zeros((1, D), bf16)

    in_maps = []
    for c in range(NCORES):
        b, g = divmod(c, 4)
        cs = slice(g * HPC * DH, (g + 1) * HPC * DH)
        in_maps.append({
            "xT": xTs[b],
            "wq": np.ascontiguousarray(Wq[:, cs]).astype(bf16),
            "wk": np.ascontiguousarray(Wk[:, cs]).astype(bf16),
            "wv": np.ascontiguousarray(Wv[:, cs]).astype(bf16),
            "wo": np.ascontiguousarray(Wo[cs, :]).astype(bf16),
            "bo_r": bo_row if g == 0 else zeros_row,
            "tri": tri,
        })
    return in_maps

